# revision 1
# baseline (speedup 1.0000x reference)
"""Trainium2 Bass kernel for nn_OmegaEntangle (E^T C E with entangle coefficients).

Math (validated vs reference to ~8e-7 rel err in fp32):
  p_i = sum_j v_ij^2 ; m_i = mean_j v_ij
  C[i,j] = mask(i<j) * sqrt(p_i p_j) * (m_i + 1j*m_j) / sqrt(m_i^2 + m_j^2)
  out = E^T C E   (complex, E real)  ->  out_re = E^T Cr E, out_im = E^T Ci E

Sharding: data-parallel over the 2048 OUTPUT COLUMNS (256 per core), with the
p/m reduction row-sharded (64 rows per core).

Two NEFF launches (a device collective would cost ~60+ us of entry-barrier +
AllGather latency on this platform for 768 bytes; host concat of the tiny
reduction result is far cheaper):
  Kernel A: each core reduces its [64, 32768] vuln shard -> p[64], msum[64].
  Host: concatenates the 8 shards (pure data movement, no math).
  Kernel B: each core derives sp/a/m2 vectors, builds C^T, computes
    T = C @ E[:, cols] and out[:, cols] = E^T @ T, writes [2048, 256] slabs.
Host concatenates slabs along columns -> [2048, 2048] complex64.
"""

import numpy as np

import concourse.bass as bass
import concourse.mybir as mybir
import concourse.tile as tile
from concourse import bacc
from concourse.bass_utils import run_bass_kernel_spmd

D = 512          # number of domains
V = 32768        # vuln dim
S = 2048         # sup (embed) dim
NCORES = 8
ROWS_PER_CORE = D // NCORES          # 64
COLS_PER_CORE = S // NCORES          # 256
NVT = 8                               # number of vuln tiles per core
VFREE = (ROWS_PER_CORE * V) // (128 * NVT)   # 2048 free elems per vuln tile
KT = D // 128                         # 4 contraction tiles
MT = S // 128                         # 16 output row tiles
INV_V = 1.0 / V
WARMUP_MMS = 36                       # PE warm-up matmuls at kernel-B start

F32 = mybir.dt.float32
F32R = mybir.dt.float32r
BF16 = mybir.dt.bfloat16
# float32r (TF32) matmul inputs stream at 1 cyc/row vs 4 for float32.
# Host pre-rounds E to TF32 values; on-device producers of matmul operands
# write float32r-typed tiles so the BIR verifier sees rounded inputs.


def _tf32_round(x):
    xi = np.ascontiguousarray(x, dtype=np.float32).view(np.uint32)
    return ((xi + np.uint32(0x1000)) & np.uint32(0xFFFFE000)).view(np.float32)
AF = mybir.ActivationFunctionType
ALU = mybir.AluOpType

_CACHE = {}


def build_kernel_a():
    """Reduce kernel: per-core p/msum over the 64-row vuln shard."""
    nc = bacc.Bacc("TRN2", target_bir_lowering=False, debug=False, num_devices=NCORES)

    v128 = nc.dram_tensor("v128", [128, NVT * VFREE], F32, kind="ExternalInput")
    pairmat = nc.dram_tensor("pairmat", [128, ROWS_PER_CORE], F32, kind="ExternalInput")
    out_pm = nc.dram_tensor("out_pm", [ROWS_PER_CORE, 2], F32, kind="ExternalOutput")
    widths = [2048] * 6 + [1024] * 4

    with tile.TileContext(nc) as tc:
        with (
            tc.tile_pool(name="vin", bufs=3) as vin_pool,
            tc.tile_pool(name="scr", bufs=2) as scr_pool,
            tc.tile_pool(name="small", bufs=1) as small_pool,
            tc.tile_pool(name="ps", bufs=1, space="PSUM") as ps_pool,
        ):
            vts = []
            off = 0
            for t, w in enumerate(widths):
                vt = vin_pool.tile([128, VFREE], F32, name=f"vt{t}", tag="vt")
                nc.sync.dma_start(vt[:, 0:w], v128[:, off : off + w])
                off += w
                vts.append(vt)
            pair_sb = small_pool.tile([128, ROWS_PER_CORE], F32, name="pair_sb")
            nc.sync.dma_start(pair_sb[:], pairmat[:])

            NT = len(widths)
            pm_acc = small_pool.tile([128, 2 * NT], F32, name="pm_acc")
            for t, w in enumerate(widths):
                sq = scr_pool.tile([128, VFREE], F32, name="sq", tag="sq")
                nc.scalar.activation(
                    sq[:, 0:w], vts[t][:, 0:w], AF.Square,
                    accum_out=pm_acc[:, t : t + 1],
                )
                raw = scr_pool.tile([128, VFREE], F32, name="raw", tag="raw")
                nc.vector.tensor_scalar(
                    raw[:, 0:w], vts[t][:, 0:w], 1.0, None, ALU.mult, ALU.add,
                    accum_out=pm_acc[:, NT + t : NT + t + 1],
                )

            ps_pm = ps_pool.tile([ROWS_PER_CORE, 2 * NT], F32, name="ps_pm")
            nc.tensor.matmul(ps_pm[:], pair_sb[:], pm_acc[:], start=True, stop=True)

            d2 = small_pool.tile([ROWS_PER_CORE, 2], F32, name="d2")
            nc.vector.tensor_reduce(
                d2[:, 0:1], ps_pm[:, 0:NT], mybir.AxisListType.X, ALU.add
            )
            nc.vector.tensor_reduce(
                d2[:, 1:2], ps_pm[:, NT : 2 * NT], mybir.AxisListType.X, ALU.add
            )
            nc.sync.dma_start(out_pm[:], d2[:])

    nc.compile()
    return nc


def build_kernel_b():
    """Main kernel: derive vectors, build C^T, two matmul chains, write slab."""
    nc = bacc.Bacc("TRN2", target_bir_lowering=False, debug=False, num_devices=NCORES)

    # pm_pp: per-partition layout, col kt   = p[q + 128*kt],
    #        col 4+kt = msum[q + 128*kt]    (q = partition)
    pm_pp = nc.dram_tensor("pm_pp", [128, 2 * KT], F32, kind="ExternalInput")
    # raw reduction outputs replicated across partitions (host-side replication)
    p_bc_in = nc.dram_tensor("p_bc", [128, D], F32, kind="ExternalInput")
    ms_bc_in = nc.dram_tensor("ms_bc", [128, D], F32, kind="ExternalInput")
    efull = nc.dram_tensor("efull", [KT, 128, S], F32R, kind="ExternalInput")
    ecols = nc.dram_tensor("ecols", [KT, 128, COLS_PER_CORE], F32R, kind="ExternalInput")
    # transposed output slabs: host transposes back (out[:, cols] = slab.T)
    out_re = nc.dram_tensor("out_re", [COLS_PER_CORE, S], F32, kind="ExternalOutput")
    out_im = nc.dram_tensor("out_im", [COLS_PER_CORE, S], F32, kind="ExternalOutput")

    with tile.TileContext(nc) as tc:
        with (
            tc.tile_pool(name="epool", bufs=1) as e_pool,
            tc.tile_pool(name="small", bufs=1) as small_pool,
            tc.tile_pool(name="cbuild", bufs=2) as cb_pool,
            tc.tile_pool(name="ctp", bufs=1) as ct_pool,
            tc.tile_pool(name="tsb", bufs=1) as t_pool,
            tc.tile_pool(name="ost", bufs=4) as o_pool,
            tc.tile_pool(name="psA", bufs=4, space="PSUM") as psA,
            tc.tile_pool(name="psB", bufs=4, space="PSUM") as psB,
        ):
            # -------- input DMAs (small first, then E) ------------------------
            pp = small_pool.tile([128, 2 * KT], F32, name="pp")
            nc.sync.dma_start(pp[:], pm_pp[:])
            p_bct = small_pool.tile([128, D], F32, name="p_bct")
            nc.sync.dma_start(p_bct[:], p_bc_in[:])
            ms_bct = small_pool.tile([128, D], F32, name="ms_bct")
            nc.sync.dma_start(ms_bct[:], ms_bc_in[:])

            ec_sb = []
            for kt in range(KT):
                ect = e_pool.tile(
                    [128, COLS_PER_CORE], F32R, name=f"ec{kt}", tag=f"ec{kt}"
                )
                nc.sync.dma_start(ect[:], ecols[kt])
                ec_sb.append(ect)
            e_sb = []
            for kt in range(KT):
                et = e_pool.tile([128, S], F32R, name=f"e{kt}", tag=f"e{kt}")
                nc.sync.dma_start(et[:], efull[kt])
                e_sb.append(et)

            # -------- PE warm-up during the small-vector derivation -----------
            warm_b = small_pool.tile([128, 512], BF16, name="warm_b")
            nc.gpsimd.memset(warm_b[:], 0.001)
            ps_w = psB.tile([128, 512], F32, name="ps_w", tag="o")
            for i in range(WARMUP_MMS):
                nc.tensor.matmul(
                    ps_w[:], warm_b[:, 0:128], warm_b[:],
                    start=(i == 0), stop=(i == WARMUP_MMS - 1),
                )

            # -------- derived vectors -----------------------------------------
            # per-partition [128, 4] each
            sp4 = small_pool.tile([128, KT], F32, name="sp4")
            a4 = small_pool.tile([128, KT], F32, name="a4")
            m24 = small_pool.tile([128, KT], F32, name="m24")
            nc.vector.scalar_tensor_tensor(
                m24[:], pp[:, KT : 2 * KT], INV_V * INV_V, pp[:, KT : 2 * KT],
                op0=ALU.mult, op1=ALU.mult,
            )
            nc.scalar.activation(sp4[:], pp[:, 0:KT], AF.Sqrt)
            nc.vector.scalar_tensor_tensor(
                a4[:], pp[:, KT : 2 * KT], INV_V, sp4[:], op0=ALU.mult, op1=ALU.mult
            )
            # broadcast derived tiles straight from the replicated raw inputs
            sp_bc = small_pool.tile([128, D], F32, name="sp_bc")
            a_bc = small_pool.tile([128, D], F32, name="a_bc")
            m2_bc = small_pool.tile([128, D], F32, name="m2_bc")
            nc.vector.scalar_tensor_tensor(
                m2_bc[:], ms_bct[:], INV_V * INV_V, ms_bct[:],
                op0=ALU.mult, op1=ALU.mult,
            )
            nc.scalar.activation(sp_bc[:], p_bct[:], AF.Sqrt)
            nc.vector.scalar_tensor_tensor(
                a_bc[:], ms_bct[:], INV_V, sp_bc[:], op0=ALU.mult, op1=ALU.mult
            )

            # -------- build C^T (real and imag) -------------------------------
            ct_r, ct_i = [], []
            for jt in range(KT):
                h = cb_pool.tile([128, D], F32, name="h", tag="h")
                nc.scalar.activation(
                    h[:], m2_bc[:], AF.Sqrt, bias=m24[:, jt : jt + 1], scale=1.0
                )
                rinv = cb_pool.tile([128, D], F32, name="rinv", tag="rinv")
                nc.vector.reciprocal_approx_fast(out=rinv[:], in_=h[:])
                rm = cb_pool.tile([128, D], F32, name="rm", tag="rm")
                nc.gpsimd.affine_select(
                    out=rm[:], in_=rinv[:],
                    pattern=[[-1, D]], compare_op=ALU.is_gt,
                    fill=0.0, base=128 * jt, channel_multiplier=1,
                )
                ctr = ct_pool.tile([128, D], F32R, name=f"ctr{jt}", tag=f"ctr{jt}")
                cti = ct_pool.tile([128, D], F32R, name=f"cti{jt}", tag=f"cti{jt}")
                nc.vector.scalar_tensor_tensor(
                    ctr[:], a_bc[:], sp4[:, jt : jt + 1], rm[:],
                    op0=ALU.mult, op1=ALU.mult,
                )
                nc.vector.scalar_tensor_tensor(
                    cti[:], sp_bc[:], a4[:, jt : jt + 1], rm[:],
                    op0=ALU.mult, op1=ALU.mult,
                )
                ct_r.append(ctr)
                ct_i.append(cti)

            # -------- T = C @ E[:, cols]  ([128, 512] = [T_r | T_i]) ----------
            ps_ts = [
                psA.tile(
                    [128, 2 * COLS_PER_CORE], F32, name=f"ps_t{it}", tag=f"t{it}",
                    bufs=1,
                )
                for it in range(KT)
            ]
            for part, cts in ((0, ct_r), (1, ct_i)):
                lo = part * COLS_PER_CORE
                for jt in range(KT):
                    for it in range(KT):
                        nc.tensor.matmul(
                            ps_ts[it][:, lo : lo + COLS_PER_CORE],
                            cts[jt][:, it * 128 : (it + 1) * 128],
                            ec_sb[jt][:],
                            start=(jt == 0), stop=(jt == KT - 1),
                        )
                    if part == 0 and jt == 1:
                        # filler burst: keep the PE busy while the C-build
                        # produces the last two tiles (avoids a HAM re-throttle)
                        for i in range(8):
                            nc.tensor.matmul(
                                ps_w[:], warm_b[:, 0:128], warm_b[:],
                                start=(i == 0), stop=(i == 7),
                            )
            t_sb = []
            for it in range(KT):
                tsb = t_pool.tile(
                    [128, 2 * COLS_PER_CORE], F32R, name=f"tsb{it}", tag=f"tsb{it}"
                )
                if it % 2 == 0:
                    nc.scalar.copy(tsb[:], ps_ts[it][:])
                else:
                    nc.vector.tensor_copy(tsb[:], ps_ts[it][:])
                t_sb.append(tsb)

            # -------- out^T[cols, :] = T^T @ E  (transposed chain) ------------
            # lhsT = T[i, c] slices straight from t_sb; rhs = e_sb 512-chunks.
            # Consecutive sn-matmuls share the same stationary operand.
            NS = S // 512
            cnt = 0
            for part, outT in ((0, out_re), (1, out_im)):
                for mc in range(2):
                    c0 = part * COLS_PER_CORE + mc * 128
                    pso = [
                        psB.tile([128, 512], F32, name=f"pso{sn}", tag="o")
                        for sn in range(NS)
                    ]
                    for it in range(KT):
                        for sn in range(NS):
                            nc.tensor.matmul(
                                pso[sn][:],
                                t_sb[it][:, c0 : c0 + 128],
                                e_sb[it][:, sn * 512 : (sn + 1) * 512],
                                start=(it == 0), stop=(it == KT - 1),
                            )
                    for sn in range(NS):
                        osb = o_pool.tile([128, 512], F32, name="osb", tag="osb")
                        if cnt % 2 == 0:
                            nc.scalar.copy(osb[:], pso[sn][:])
                        else:
                            nc.vector.tensor_copy(osb[:], pso[sn][:])
                        eng = nc.sync if cnt % 2 == 0 else nc.scalar
                        eng.dma_start(
                            outT[mc * 128 : (mc + 1) * 128, sn * 512 : (sn + 1) * 512],
                            osb[:],
                        )
                        cnt += 1

    nc.compile()
    return nc


def _prepare_a_in_maps(vulns):
    vulns = np.ascontiguousarray(np.asarray(vulns, dtype=np.float32))
    pair = np.ascontiguousarray(
        np.repeat(np.eye(ROWS_PER_CORE, dtype=np.float32), 2, axis=0)
    )
    in_maps = []
    for c in range(NCORES):
        vsh = vulns[c * ROWS_PER_CORE : (c + 1) * ROWS_PER_CORE]
        in_maps.append(
            {
                "v128": np.ascontiguousarray(vsh.reshape(128, NVT * VFREE)),
                "pairmat": pair,
            }
        )
    return in_maps


def _prepare_b_in_maps(embed_table, domain_ids, p_full, msum_full):
    embed_table = np.ascontiguousarray(np.asarray(embed_table, dtype=np.float32))
    domain_ids = np.asarray(domain_ids).astype(np.int64)
    E = np.ascontiguousarray(embed_table[domain_ids])  # [512, 2048]
    e4 = _tf32_round(E).reshape(KT, 128, S)
    # per-partition layout [128, 8]
    pm_pp = np.empty((128, 2 * KT), dtype=np.float32)
    pm_pp[:, 0:KT] = p_full.reshape(KT, 128).T
    pm_pp[:, KT : 2 * KT] = msum_full.reshape(KT, 128).T
    p_bc = np.ascontiguousarray(
        np.broadcast_to(p_full.astype(np.float32), (128, D))
    )
    ms_bc = np.ascontiguousarray(
        np.broadcast_to(msum_full.astype(np.float32), (128, D))
    )
    in_maps = []
    for c in range(NCORES):
        in_maps.append(
            {
                "pm_pp": pm_pp,
                "p_bc": p_bc,
                "ms_bc": ms_bc,
                "efull": e4,
                "ecols": np.ascontiguousarray(
                    e4[:, :, c * COLS_PER_CORE : (c + 1) * COLS_PER_CORE]
                ),
            }
        )
    return in_maps


def kernel(vulns, embed_table, domain_ids, _trace=False):
    if "nc_a" not in _CACHE:
        _CACHE["nc_a"] = build_kernel_a()
    if "nc_b" not in _CACHE:
        _CACHE["nc_b"] = build_kernel_b()

    res_a = run_bass_kernel_spmd(
        _CACHE["nc_a"], _prepare_a_in_maps(vulns),
        core_ids=list(range(NCORES)), trace=_trace,
    )
    _CACHE["res_a"] = res_a
    p_full = np.concatenate([res_a.results[c]["out_pm"][:, 0] for c in range(NCORES)])
    msum_full = np.concatenate(
        [res_a.results[c]["out_pm"][:, 1] for c in range(NCORES)]
    )

    res_b = run_bass_kernel_spmd(
        _CACHE["nc_b"], _prepare_b_in_maps(embed_table, domain_ids, p_full, msum_full),
        core_ids=list(range(NCORES)), trace=_trace,
    )
    _CACHE["res_b"] = res_b

    out = np.empty((S, S), dtype=np.complex64)
    for c in range(NCORES):
        r = res_b.results[c]
        sl = slice(c * COLS_PER_CORE, (c + 1) * COLS_PER_CORE)
        out[:, sl] = r["out_re"].T + 1j * r["out_im"].T
    return out


if __name__ == "__main__":
    rng = np.random.default_rng(0)
    v = rng.standard_normal((D, V), dtype=np.float32)
    et = rng.standard_normal((D, S), dtype=np.float32)
    ids = np.arange(D, dtype=np.int32)
    out = kernel(v, et, ids)
    print(out.shape, out.dtype)



# revision 4
# speedup vs baseline: 1.0871x; 1.0871x over previous
"""Trainium2 Bass kernel for nn_OmegaEntangle (E^T C E with entangle coefficients).

Math (validated vs reference, ~5e-3 rel err in bf16):
  p_i = sum_j v_ij^2 ; msum_i = sum_j v_ij ; m_i = msum_i / V
  C[i,j] = mask(i<j) * sqrt(p_i p_j) * (m_i + 1j*m_j) / sqrt(m_i^2 + m_j^2)
  out = E^T C E   (complex, E real)  ->  out_re = E^T Cr E, out_im = E^T Ci E

Key decomposition: with a_i = m_i*sqrt(p_i), b_i = sqrt(p_i) and
R_ij = mask(i<j)/sqrt(msum_i^2+msum_j^2)  (so  r~_ij = V * R_ij):
  Cr = diag(a) r~ diag(b)  ->  T_re = a ⊙ (R @ (E · bV)),  bV_j = V*sqrt(p_j)
  Ci = diag(b) r~ diag(a)  ->  T_im = b ⊙ (R @ (E · aV)),  aV_j = msum_j*sqrt(p_j)
  out_re = E^T T_re ; out_im = E^T T_im
Only ONE real-valued masked matrix R is built (4 [128,512] bf16 tiles); the
diag scalings fold into cheap per-partition tensor_scalar ops. Only the 4
diagonal blocks of R need the triangular mask (upper off-diagonal blocks are
dense, lower blocks are skipped entirely -> 10 block-matmuls in chain 1).

All matmul operands and big DMA payloads are bf16 (host-cast; tolerance is
2e-2, this lands ~5e-3).

Sharding: data-parallel over the 2048 OUTPUT COLUMNS (256 per core), with the
p/m reduction row-sharded (64 rows per core). Two NEFF launches with a host
gather of the tiny [512,2] reduction result between them.
"""

import numpy as np
import ml_dtypes

import concourse.bass as bass
import concourse.mybir as mybir
import concourse.tile as tile
from concourse import bacc
from concourse.bass_utils import run_bass_kernel_spmd
from concourse.dve_ops import RECIP_APPROX_FAST_CONSTS, RECIPROCAL_APPROX_FAST

D = 512          # number of domains
V = 32768        # vuln dim
S = 2048         # sup (embed) dim
NCORES = 8
ROWS_PER_CORE = D // NCORES          # 64
COLS_PER_CORE = S // NCORES          # 256
KT = D // 128                         # 4 contraction blocks
NS = S // 512                         # 4 chain-2 output column chunks
INV_V = 1.0 / V
BF = ml_dtypes.bfloat16

F32 = mybir.dt.float32
BF16 = mybir.dt.bfloat16
AF = mybir.ActivationFunctionType
ALU = mybir.AluOpType

# ---- tunables -------------------------------------------------------------
A_WIDTHS = [2048] * 7 + [1024, 512, 512]      # 16384 total bf16 elems/partition
A_BUFS = 4
B_WARMUP = 16                                  # PE warm-up matmuls (HAM ramp)
B_WARM_FREE = 128

_CACHE = {}


def build_kernel_a():
    """Reduce kernel: per-core p/msum over the 64-row vuln shard (bf16 input)."""
    nc = bacc.Bacc("TRN2", target_bir_lowering=False, debug=False, num_devices=NCORES)

    v128 = nc.dram_tensor("v128", [128, 16384], BF16, kind="ExternalInput")
    pairmat = nc.dram_tensor("pairmat", [128, ROWS_PER_CORE], F32, kind="ExternalInput")
    out_pm = nc.dram_tensor("out_pm", [ROWS_PER_CORE, 2], F32, kind="ExternalOutput")

    NT = len(A_WIDTHS)
    with tile.TileContext(nc) as tc:
        with (
            tc.tile_pool(name="vin", bufs=A_BUFS) as vin_pool,
            tc.tile_pool(name="scr", bufs=2) as scr_pool,
            tc.tile_pool(name="small", bufs=1) as small_pool,
            tc.tile_pool(name="ps", bufs=1, space="PSUM") as ps_pool,
        ):
            pair_sb = small_pool.tile([128, ROWS_PER_CORE], F32, name="pair_sb")
            nc.sync.dma_start(pair_sb[:], pairmat[:])
            vts = []
            off = 0
            for t, w in enumerate(A_WIDTHS):
                vt = vin_pool.tile([128, 2048], BF16, name=f"vt{t}", tag="vt")
                nc.sync.dma_start(vt[:, 0:w], v128[:, off : off + w])
                off += w
                vts.append(vt)

            pm_acc = small_pool.tile([128, 2 * NT], F32, name="pm_acc")
            for t, w in enumerate(A_WIDTHS):
                sq = scr_pool.tile([128, 2048], BF16, name="sq", tag="sq")
                nc.scalar.activation(
                    sq[:, 0:w], vts[t][:, 0:w], AF.Square,
                    accum_out=pm_acc[:, t : t + 1],
                )
                raw = scr_pool.tile([128, 2048], BF16, name="raw", tag="raw")
                nc.vector.tensor_scalar(
                    raw[:, 0:w], vts[t][:, 0:w], 1.0, None, ALU.mult, ALU.add,
                    accum_out=pm_acc[:, NT + t : NT + t + 1],
                )

            ps_pm = ps_pool.tile([ROWS_PER_CORE, 2 * NT], F32, name="ps_pm")
            nc.tensor.matmul(ps_pm[:], pair_sb[:], pm_acc[:], start=True, stop=True)

            d2 = small_pool.tile([ROWS_PER_CORE, 2], F32, name="d2")
            nc.vector.tensor_reduce(
                d2[:, 0:1], ps_pm[:, 0:NT], mybir.AxisListType.X, ALU.add
            )
            nc.vector.tensor_reduce(
                d2[:, 1:2], ps_pm[:, NT : 2 * NT], mybir.AxisListType.X, ALU.add
            )
            nc.sync.dma_start(out_pm[:], d2[:])

    nc.compile()
    return nc


def build_kernel_b():
    """Main kernel: build R, two matmul chains, write transposed bf16 slabs."""
    nc = bacc.Bacc("TRN2", target_bir_lowering=False, debug=False, num_devices=NCORES)

    # pm_pp: per-partition layout, col kt = p[q+128kt], 4+kt = msum, 8+kt = msum^2
    pm_pp = nc.dram_tensor("pm_pp", [128, 3 * KT], F32, kind="ExternalInput")
    # msum^2 replicated across partitions (host-side replication), bf16
    ms2_bc_in = nc.dram_tensor("ms2_bc", [128, D], BF16, kind="ExternalInput")
    # E column shard [128, KT*256]: j-block jt at cols [256*jt, 256*(jt+1))
    ecols = nc.dram_tensor("ecols", [128, KT * COLS_PER_CORE], BF16, kind="ExternalInput")
    efull = nc.dram_tensor("efull", [KT, 128, S], BF16, kind="ExternalInput")
    # transposed output slabs (host transposes back): out[:, cols] = slab.T
    out_re = nc.dram_tensor("out_re", [COLS_PER_CORE, S], BF16, kind="ExternalOutput")
    out_im = nc.dram_tensor("out_im", [COLS_PER_CORE, S], BF16, kind="ExternalOutput")

    rc = RECIP_APPROX_FAST_CONSTS
    CP = COLS_PER_CORE

    with tile.TileContext(nc) as tc:
        with (
            tc.tile_pool(name="epool", bufs=1) as e_pool,
            tc.tile_pool(name="small", bufs=1) as small_pool,
            tc.tile_pool(name="hb", bufs=2) as h_pool,
            tc.tile_pool(name="ost", bufs=4) as o_pool,
            tc.tile_pool(name="psA", bufs=1, space="PSUM") as psA,
            tc.tile_pool(name="psB", bufs=4, space="PSUM") as psB,
        ):
            # -------- input DMAs --------------------------------------------
            # sync: small early tensors; gpsimd: e0/e1; scalar (after h's): e2/e3
            pp = small_pool.tile([128, 3 * KT], F32, name="pp")
            nc.sync.dma_start(pp[:], pm_pp[:])
            ms2_bc = small_pool.tile([128, D], BF16, name="ms2_bc")
            nc.sync.dma_start(ms2_bc[:], ms2_bc_in[:])
            ec_sb = small_pool.tile([128, KT * CP], BF16, name="ec_sb")
            nc.sync.dma_start(ec_sb[:], ecols[:])

            e_sb = [
                e_pool.tile([128, S], BF16, name=f"e{kt}", tag=f"e{kt}")
                for kt in range(KT)
            ]

            # -------- PE warm-up (HAM ramp) during DMA/derivation -----------
            warm_b = small_pool.tile([128, B_WARM_FREE], BF16, name="warm_b")
            nc.gpsimd.memset(warm_b[:], 0.001)
            nc.gpsimd.dma_start(e_sb[0][:], efull[0])
            nc.gpsimd.dma_start(e_sb[1][:], efull[1])
            ps_w = psB.tile([128, 512], F32, name="ps_w", tag="o")
            for i in range(B_WARMUP):
                nc.tensor.matmul(
                    ps_w[:, 0:B_WARM_FREE], warm_b[:], warm_b[:],
                    start=(i == 0), stop=(i == B_WARMUP - 1),
                )

            # -------- tiny derived vectors ----------------------------------
            # b4n = sqrt(p); bv4 = V*sqrt(p); av4 = msum*sqrt(p); a4c = av4/V
            b4n = small_pool.tile([128, KT], F32, name="b4n")
            nc.scalar.activation(b4n[:], pp[:, 0:KT], AF.Sqrt)
            bv4 = small_pool.tile([128, KT], F32, name="bv4")
            nc.vector.tensor_scalar_mul(bv4[:], b4n[:], float(V))
            av4 = small_pool.tile([128, KT], F32, name="av4")
            nc.vector.scalar_tensor_tensor(
                av4[:], pp[:, KT : 2 * KT], 1.0, b4n[:], op0=ALU.mult, op1=ALU.mult
            )
            a4c = small_pool.tile([128, KT], F32, name="a4c")
            nc.vector.tensor_scalar_mul(a4c[:], av4[:], INV_V)

            # -------- R build + scaled-E operand prep -----------------------
            rt, rd, ebea = [], [], []
            for jt in range(KT):
                h = h_pool.tile([128, D], F32, name="h", tag="h")
                nc.scalar.activation(
                    h[:], ms2_bc[:], AF.Sqrt,
                    bias=pp[:, 2 * KT + jt : 2 * KT + jt + 1], scale=1.0,
                )
                rtj = e_pool.tile([128, D], BF16, name=f"rt{jt}", tag=f"rt{jt}")
                nc.vector._custom_dve(
                    RECIPROCAL_APPROX_FAST, out=rtj[:], in0=h[:],
                    s0=rc["s0"], s1=rc["s1"], imm2=rc["imm2"],
                )
                rt.append(rtj)
                # strictly-upper mask for the diagonal block only
                rdj = e_pool.tile([128, 128], BF16, name=f"rd{jt}", tag=f"rd{jt}")
                nc.gpsimd.affine_select(
                    out=rdj[:], in_=rtj[:, 128 * jt : 128 * (jt + 1)],
                    pattern=[[-1, 128]], compare_op=ALU.is_gt,
                    fill=0.0, base=0, channel_multiplier=1,
                )
                rd.append(rdj)
                # ebea[jt] = [E·bV | E·aV] for this j-block (bf16, 4x DVE mode)
                ebj = e_pool.tile([128, 2 * CP], BF16, name=f"eb{jt}", tag=f"eb{jt}")
                nc.vector.tensor_scalar_mul(
                    ebj[:, 0:CP], ec_sb[:, CP * jt : CP * (jt + 1)],
                    bv4[:, jt : jt + 1],
                )
                nc.vector.tensor_scalar_mul(
                    ebj[:, CP : 2 * CP], ec_sb[:, CP * jt : CP * (jt + 1)],
                    av4[:, jt : jt + 1],
                )
                ebea.append(ebj)

            # remaining big-E DMAs issued from the scalar queue after its h's
            nc.scalar.dma_start(e_sb[2][:], efull[2])
            nc.scalar.dma_start(e_sb[3][:], efull[3])

            # -------- chain 1: T-blocks = R @ [E·bV | E·aV] -----------------
            ps_ts = [
                psA.tile([128, 2 * CP], F32, name=f"ps_t{it}", tag=f"t{it}", bufs=1)
                for it in range(KT)
            ]
            # jt-outer so each rt/rd/ebea tile is consumed as soon as ready;
            # diag block (it==jt) opens each accumulation group.
            for jt in range(KT):
                nc.tensor.matmul(
                    ps_ts[jt][:], rd[jt][:], ebea[jt][:],
                    start=True, stop=(jt == KT - 1),
                )
                for it in range(jt):
                    nc.tensor.matmul(
                        ps_ts[it][:], rt[jt][:, 128 * it : 128 * (it + 1)], ebea[jt][:],
                        start=False, stop=(jt == KT - 1),
                    )

            # -------- T -> SBUF with diag scalings (bf16) -------------------
            # t_sb[it][:, 0:256] = a ⊙ (R Eb) = T_re ; [:, 256:512] = b ⊙ (R Ea) = T_im
            t_sb = []
            for it in range(KT):
                tsb = e_pool.tile([128, 2 * CP], BF16, name=f"tsb{it}", tag=f"tsb{it}")
                nc.scalar.activation(
                    tsb[:, 0:CP], ps_ts[it][:, 0:CP], AF.Copy,
                    scale=a4c[:, it : it + 1],
                )
                nc.vector.tensor_scalar_mul(
                    tsb[:, CP : 2 * CP], ps_ts[it][:, CP : 2 * CP],
                    b4n[:, it : it + 1],
                )
                t_sb.append(tsb)

            # -------- chain 2: out^T slabs = T^T @ E ------------------------
            # groups: (re, mc0), (re, mc1), (im, mc0), (im, mc1)
            groups = [
                (0, 0, out_re), (0, 1, out_re), (1, 0, out_im), (1, 1, out_im),
            ]
            cnt = 0
            for part, mc, out_t in groups:
                c0 = part * CP + mc * 128
                pso = [
                    psB.tile([128, 512], F32, name=f"pso{part}{mc}{sn}", tag="o")
                    for sn in range(NS)
                ]
                for it in range(KT):
                    for sn in range(NS):
                        nc.tensor.matmul(
                            pso[sn][:],
                            t_sb[it][:, c0 : c0 + 128],
                            e_sb[it][:, 512 * sn : 512 * (sn + 1)],
                            start=(it == 0), stop=(it == KT - 1),
                        )
                for sn in range(NS):
                    osb = o_pool.tile([128, 512], BF16, name="osb", tag="osb")
                    if cnt % 2 == 0:
                        nc.scalar.copy(osb[:], pso[sn][:])
                    else:
                        nc.vector.tensor_copy(osb[:], pso[sn][:])
                    nc.sync.dma_start(
                        out_t[mc * 128 : (mc + 1) * 128, 512 * sn : 512 * (sn + 1)],
                        osb[:],
                    )
                    cnt += 1

    nc.compile()
    return nc


def _prepare_a_in_maps(vulns):
    vulns = np.asarray(vulns)
    pair = np.ascontiguousarray(
        np.repeat(np.eye(ROWS_PER_CORE, dtype=np.float32), 2, axis=0)
    )
    in_maps = []
    for c in range(NCORES):
        vsh = vulns[c * ROWS_PER_CORE : (c + 1) * ROWS_PER_CORE]
        v128 = np.ascontiguousarray(
            vsh.astype(BF).reshape(128, 16384)
        )
        in_maps.append({"v128": v128, "pairmat": pair})
    return in_maps


def _prepare_b_in_maps(embed_table, domain_ids, p_full, msum_full):
    embed_table = np.asarray(embed_table, dtype=np.float32)
    domain_ids = np.asarray(domain_ids).astype(np.int64)
    E = np.ascontiguousarray(embed_table[domain_ids])          # [512, 2048] f32
    Ebf = E.astype(BF)
    e4 = np.ascontiguousarray(Ebf.reshape(KT, 128, S))
    ms2 = (msum_full.astype(np.float64) ** 2).astype(np.float32)
    # per-partition layout [128, 12]
    pm_pp = np.empty((128, 3 * KT), dtype=np.float32)
    pm_pp[:, 0:KT] = p_full.reshape(KT, 128).T
    pm_pp[:, KT : 2 * KT] = msum_full.reshape(KT, 128).T
    pm_pp[:, 2 * KT : 3 * KT] = ms2.reshape(KT, 128).T
    ms2_bc = np.ascontiguousarray(
        np.broadcast_to(ms2.astype(BF), (128, D))
    )
    in_maps = []
    for c in range(NCORES):
        cols = slice(c * COLS_PER_CORE, (c + 1) * COLS_PER_CORE)
        ecols = np.ascontiguousarray(
            e4[:, :, cols].transpose(1, 0, 2).reshape(128, KT * COLS_PER_CORE)
        )
        in_maps.append(
            {"pm_pp": pm_pp, "ms2_bc": ms2_bc, "ecols": ecols, "efull": e4}
        )
    return in_maps


def kernel(vulns, embed_table, domain_ids, _trace=False):
    if "nc_a" not in _CACHE:
        _CACHE["nc_a"] = build_kernel_a()
    if "nc_b" not in _CACHE:
        _CACHE["nc_b"] = build_kernel_b()

    res_a = run_bass_kernel_spmd(
        _CACHE["nc_a"], _prepare_a_in_maps(vulns),
        core_ids=list(range(NCORES)), trace=_trace,
    )
    _CACHE["res_a"] = res_a
    p_full = np.concatenate([res_a.results[c]["out_pm"][:, 0] for c in range(NCORES)])
    msum_full = np.concatenate(
        [res_a.results[c]["out_pm"][:, 1] for c in range(NCORES)]
    )

    res_b = run_bass_kernel_spmd(
        _CACHE["nc_b"], _prepare_b_in_maps(embed_table, domain_ids, p_full, msum_full),
        core_ids=list(range(NCORES)), trace=_trace,
    )
    _CACHE["res_b"] = res_b

    out = np.empty((S, S), dtype=np.complex64)
    for c in range(NCORES):
        r = res_b.results[c]
        sl = slice(c * COLS_PER_CORE, (c + 1) * COLS_PER_CORE)
        out[:, sl] = (
            r["out_re"].astype(np.float32).T
            + 1j * r["out_im"].astype(np.float32).T
        )
    return out


if __name__ == "__main__":
    rng = np.random.default_rng(0)
    v = rng.standard_normal((D, V), dtype=np.float32)
    et = rng.standard_normal((D, S), dtype=np.float32)
    ids = np.arange(D, dtype=np.int32)
    out = kernel(v, et, ids)
    print(out.shape, out.dtype)


# revision 6
# speedup vs baseline: 1.1366x; 1.0455x over previous
"""Trainium2 Bass kernel for nn_OmegaEntangle (E^T C E with entangle coefficients).

Math (validated vs reference, ~5e-3 rel err in bf16):
  p_i = sum_j v_ij^2 ; msum_i = sum_j v_ij ; m_i = msum_i / V
  C[i,j] = mask(i<j) * sqrt(p_i p_j) * (m_i + 1j*m_j) / sqrt(m_i^2 + m_j^2)
  out = E^T C E   (complex, E real)  ->  out_re = E^T Cr E, out_im = E^T Ci E

Key decomposition: with a_i = m_i*sqrt(p_i), b_i = sqrt(p_i) and
R_ij = mask(i<j)/sqrt(msum_i^2+msum_j^2)  (so  r~_ij = V * R_ij):
  Cr = diag(a) r~ diag(b)  ->  T_re = a ⊙ (R @ (E · bV)),  bV_j = V*sqrt(p_j)
  Ci = diag(b) r~ diag(a)  ->  T_im = b ⊙ (R @ (E · aV)),  aV_j = msum_j*sqrt(p_j)
  out_re = E^T T_re ; out_im = E^T T_im
Only ONE real-valued masked matrix R is built (4 [128,512] bf16 tiles); the
diag scalings fold into cheap per-partition tensor_scalar ops. Only the 4
diagonal blocks of R need the triangular mask (upper off-diagonal blocks are
dense, lower blocks are skipped entirely -> 10 block-matmuls in chain 1).

All matmul operands and big DMA payloads are bf16 (host-cast; tolerance is
2e-2, this lands ~5e-3).

Sharding: data-parallel over the 2048 OUTPUT COLUMNS (256 per core), with the
p/m reduction row-sharded (64 rows per core). Two NEFF launches with a host
gather of the tiny [512,2] reduction result between them.
"""

import numpy as np
import ml_dtypes

import concourse.bass as bass
import concourse.mybir as mybir
import concourse.tile as tile
from concourse import bacc
from concourse.bass_utils import run_bass_kernel_spmd
from concourse.dve_ops import RECIP_APPROX_FAST_CONSTS, RECIPROCAL_APPROX_FAST

D = 512          # number of domains
V = 32768        # vuln dim
S = 2048         # sup (embed) dim
NCORES = 8
ROWS_PER_CORE = D // NCORES          # 64
COLS_PER_CORE = S // NCORES          # 256
KT = D // 128                         # 4 contraction blocks
NS = S // 512                         # 4 chain-2 output column chunks
INV_V = 1.0 / V
BF = ml_dtypes.bfloat16

F32 = mybir.dt.float32
BF16 = mybir.dt.bfloat16
AF = mybir.ActivationFunctionType
ALU = mybir.AluOpType

# ---- tunables -------------------------------------------------------------
A_NT = 8                                       # vuln tiles per core
A_BUFS = 3
A_WARMUP = 16
B_WARMUP = 16                                  # PE warm-up matmuls (HAM ramp)
B_WARM_FREE = 128

_CACHE = {}


def build_kernel_a():
    """Reduce kernel: gram-matrix trick on the tensor engine.

    Host sends the vuln shard TRANSPOSED and ones-augmented: 256 chunks of
    [128 j, 65] where cols 0:64 = v[j, row] and col 64 = 1. One long PSUM
    accumulation of chunk^T @ chunk[:, 0:64] yields [65, 64]: rows 0:64 are
    the gram matrix (diag = p), row 64 is msum. Host extracts diag/row.
    DMA-bound (~12 us); vector/scalar engines idle.
    """
    nc = bacc.Bacc("TRN2", target_bir_lowering=False, debug=False, num_devices=NCORES)

    NCH = V // 128                   # 256 chunks
    G = NCH // A_NT                  # chunks per tile
    W = G * 65                       # tile free width (bf16 elems)
    v128 = nc.dram_tensor("v128", [128, A_NT * W], BF16, kind="ExternalInput")
    out_g = nc.dram_tensor("out_g", [65, 64], F32, kind="ExternalOutput")

    with tile.TileContext(nc) as tc:
        with (
            tc.tile_pool(name="vin", bufs=A_BUFS) as vin_pool,
            tc.tile_pool(name="small", bufs=1) as small_pool,
            tc.tile_pool(name="ps", bufs=1, space="PSUM") as ps_pool,
            tc.tile_pool(name="psw", bufs=1, space="PSUM") as psw_pool,
        ):
            # PE warm-up during preamble/first-tile DMA
            warm_b = small_pool.tile([128, 64], BF16, name="warm_b")
            nc.gpsimd.memset(warm_b[:], 0.001)
            ps_w = psw_pool.tile([64, 64], F32, name="ps_w")
            for i in range(A_WARMUP):
                nc.tensor.matmul(
                    ps_w[:], warm_b[:], warm_b[:],
                    start=(i == 0), stop=(i == A_WARMUP - 1),
                )

            vts = []
            for t in range(A_NT):
                vt = vin_pool.tile([128, W], BF16, name=f"vt{t}", tag="vt")
                nc.sync.dma_start(vt[:], v128[:, t * W : (t + 1) * W])
                vts.append(vt)

            ps_g = ps_pool.tile([65, 64], F32, name="ps_g")
            for t in range(A_NT):
                for c in range(G):
                    nc.tensor.matmul(
                        ps_g[:],
                        vts[t][:, 65 * c : 65 * c + 65],
                        vts[t][:, 65 * c : 65 * c + 64],
                        start=(t == 0 and c == 0),
                        stop=(t == A_NT - 1 and c == G - 1),
                    )

            gsb = small_pool.tile([65, 64], F32, name="gsb")
            nc.vector.tensor_copy(gsb[:], ps_g[:])
            nc.sync.dma_start(out_g[:], gsb[:])

    nc.compile()
    return nc


def build_kernel_b():
    """Main kernel: build R, two matmul chains, write transposed bf16 slabs."""
    nc = bacc.Bacc("TRN2", target_bir_lowering=False, debug=False, num_devices=NCORES)

    # pm_pp: per-partition layout, col kt = p[q+128kt], 4+kt = msum, 8+kt = msum^2
    pm_pp = nc.dram_tensor("pm_pp", [128, 3 * KT], F32, kind="ExternalInput")
    # msum^2 replicated across partitions (host-side replication), bf16
    ms2_bc_in = nc.dram_tensor("ms2_bc", [128, D], BF16, kind="ExternalInput")
    # E column shard [128, KT*256]: j-block jt at cols [256*jt, 256*(jt+1))
    ecols = nc.dram_tensor("ecols", [128, KT * COLS_PER_CORE], BF16, kind="ExternalInput")
    efull = nc.dram_tensor("efull", [KT, 128, S], BF16, kind="ExternalInput")
    # transposed output slabs (host transposes back): out[:, cols] = slab.T
    out_re = nc.dram_tensor("out_re", [COLS_PER_CORE, S], BF16, kind="ExternalOutput")
    out_im = nc.dram_tensor("out_im", [COLS_PER_CORE, S], BF16, kind="ExternalOutput")

    rc = RECIP_APPROX_FAST_CONSTS
    CP = COLS_PER_CORE

    with tile.TileContext(nc) as tc:
        with (
            tc.tile_pool(name="epool", bufs=1) as e_pool,
            tc.tile_pool(name="small", bufs=1) as small_pool,
            tc.tile_pool(name="hb", bufs=2) as h_pool,
            tc.tile_pool(name="ost", bufs=4) as o_pool,
            tc.tile_pool(name="psA", bufs=1, space="PSUM") as psA,
            tc.tile_pool(name="psB", bufs=4, space="PSUM") as psB,
        ):
            # -------- input DMAs --------------------------------------------
            # sync: small early tensors; gpsimd: e0/e1; scalar (after h's): e2/e3
            pp = small_pool.tile([128, 3 * KT], F32, name="pp")
            nc.sync.dma_start(pp[:], pm_pp[:])
            ms2_bc = small_pool.tile([128, D], BF16, name="ms2_bc")
            nc.sync.dma_start(ms2_bc[:], ms2_bc_in[:])
            ec_sb = small_pool.tile([128, KT * CP], BF16, name="ec_sb")
            nc.sync.dma_start(ec_sb[:], ecols[:])

            e_sb = [
                e_pool.tile([128, S], BF16, name=f"e{kt}", tag=f"e{kt}")
                for kt in range(KT)
            ]

            # -------- PE warm-up (HAM ramp) during DMA/derivation -----------
            warm_b = small_pool.tile([128, B_WARM_FREE], BF16, name="warm_b")
            nc.gpsimd.memset(warm_b[:], 0.001)
            for kt in range(KT):
                nc.sync.dma_start(e_sb[kt][:], efull[kt])
            ps_w = psB.tile([128, 512], F32, name="ps_w", tag="o")
            for i in range(B_WARMUP):
                nc.tensor.matmul(
                    ps_w[:, 0:B_WARM_FREE], warm_b[:], warm_b[:],
                    start=(i == 0), stop=(i == B_WARMUP - 1),
                )

            # -------- tiny derived vectors ----------------------------------
            # b4n = sqrt(p); bv4 = V*sqrt(p); av4 = msum*sqrt(p); a4c = av4/V
            b4n = small_pool.tile([128, KT], F32, name="b4n")
            nc.scalar.activation(b4n[:], pp[:, 0:KT], AF.Sqrt)
            bv4 = small_pool.tile([128, KT], F32, name="bv4")
            nc.vector.tensor_scalar_mul(bv4[:], b4n[:], float(V))
            av4 = small_pool.tile([128, KT], F32, name="av4")
            nc.vector.scalar_tensor_tensor(
                av4[:], pp[:, KT : 2 * KT], 1.0, b4n[:], op0=ALU.mult, op1=ALU.mult
            )
            a4c = small_pool.tile([128, KT], F32, name="a4c")
            nc.vector.tensor_scalar_mul(a4c[:], av4[:], INV_V)

            # -------- R build + scaled-E operand prep -----------------------
            rt, rd, ebea = [], [], []
            for jt in range(KT):
                h = h_pool.tile([128, D], F32, name="h", tag="h")
                nc.scalar.activation(
                    h[:], ms2_bc[:], AF.Sqrt,
                    bias=pp[:, 2 * KT + jt : 2 * KT + jt + 1], scale=1.0,
                )
                rtj = e_pool.tile([128, D], BF16, name=f"rt{jt}", tag=f"rt{jt}")
                nc.vector._custom_dve(
                    RECIPROCAL_APPROX_FAST, out=rtj[:], in0=h[:],
                    s0=rc["s0"], s1=rc["s1"], imm2=rc["imm2"],
                )
                rt.append(rtj)
                # strictly-upper mask for the diagonal block only
                rdj = e_pool.tile([128, 128], BF16, name=f"rd{jt}", tag=f"rd{jt}")
                nc.gpsimd.affine_select(
                    out=rdj[:], in_=rtj[:, 128 * jt : 128 * (jt + 1)],
                    pattern=[[-1, 128]], compare_op=ALU.is_gt,
                    fill=0.0, base=0, channel_multiplier=1,
                )
                rd.append(rdj)
                # ebea[jt] = [E·bV | E·aV] for this j-block (bf16, 4x DVE mode)
                ebj = e_pool.tile([128, 2 * CP], BF16, name=f"eb{jt}", tag=f"eb{jt}")
                nc.vector.tensor_scalar_mul(
                    ebj[:, 0:CP], ec_sb[:, CP * jt : CP * (jt + 1)],
                    bv4[:, jt : jt + 1],
                )
                nc.vector.tensor_scalar_mul(
                    ebj[:, CP : 2 * CP], ec_sb[:, CP * jt : CP * (jt + 1)],
                    av4[:, jt : jt + 1],
                )
                ebea.append(ebj)

            # -------- chain 1: T-blocks = R @ [E·bV | E·aV] -----------------
            ps_ts = [
                psA.tile([128, 2 * CP], F32, name=f"ps_t{it}", tag=f"t{it}", bufs=1)
                for it in range(KT)
            ]
            # jt-outer so each rt/rd/ebea tile is consumed as soon as ready;
            # diag block (it==jt) opens each accumulation group.
            for jt in range(KT):
                nc.tensor.matmul(
                    ps_ts[jt][:], rd[jt][:], ebea[jt][:],
                    start=True, stop=(jt == KT - 1),
                )
                for it in range(jt):
                    nc.tensor.matmul(
                        ps_ts[it][:], rt[jt][:, 128 * it : 128 * (it + 1)], ebea[jt][:],
                        start=False, stop=(jt == KT - 1),
                    )

            # -------- T -> SBUF with diag scalings (bf16) -------------------
            # t_sb[it][:, 0:256] = a ⊙ (R Eb) = T_re ; [:, 256:512] = b ⊙ (R Ea) = T_im
            t_sb = [None] * KT
            for it in [3, 2, 1, 0]:
                tsb = e_pool.tile([128, 2 * CP], BF16, name=f"tsb{it}", tag=f"tsb{it}")
                nc.scalar.activation(
                    tsb[:, 0:CP], ps_ts[it][:, 0:CP], AF.Copy,
                    scale=a4c[:, it : it + 1],
                )
                nc.vector.tensor_scalar_mul(
                    tsb[:, CP : 2 * CP], ps_ts[it][:, CP : 2 * CP],
                    b4n[:, it : it + 1],
                )
                t_sb[it] = tsb

            # -------- chain 2: out^T slabs = T^T @ E ------------------------
            # groups: (re, mc0), (re, mc1), (im, mc0), (im, mc1)
            groups = [
                (0, 0, out_re), (0, 1, out_re), (1, 0, out_im), (1, 1, out_im),
            ]
            cnt = 0
            for part, mc, out_t in groups:
                c0 = part * CP + mc * 128
                pso = [
                    psB.tile([128, 512], F32, name=f"pso{part}{mc}{sn}", tag="o")
                    for sn in range(NS)
                ]
                for idx_it, it in enumerate([3, 2, 1, 0]):
                    for sn in range(NS):
                        nc.tensor.matmul(
                            pso[sn][:],
                            t_sb[it][:, c0 : c0 + 128],
                            e_sb[it][:, 512 * sn : 512 * (sn + 1)],
                            start=(idx_it == 0), stop=(idx_it == KT - 1),
                        )
                for sn in range(NS):
                    osb = o_pool.tile([128, 512], BF16, name="osb", tag="osb")
                    if cnt % 2 == 0:
                        nc.scalar.copy(osb[:], pso[sn][:])
                    else:
                        nc.vector.tensor_copy(osb[:], pso[sn][:])
                    nc.sync.dma_start(
                        out_t[mc * 128 : (mc + 1) * 128, 512 * sn : 512 * (sn + 1)],
                        osb[:],
                    )
                    cnt += 1

    nc.compile()
    return nc


def _prepare_a_in_maps(vulns):
    vulns = np.asarray(vulns)
    NCH = V // 128
    in_maps = []
    for c in range(NCORES):
        vsh = vulns[c * ROWS_PER_CORE : (c + 1) * ROWS_PER_CORE]
        aug = np.empty((V, 65), dtype=BF)
        aug[:, 0:64] = vsh.T.astype(BF)
        aug[:, 64] = np.asarray(1.0, dtype=BF)
        # [NCH, 128, 65] -> [128, NCH*65] with chunk c at free cols [65c, 65c+65)
        v128 = np.ascontiguousarray(
            aug.reshape(NCH, 128, 65).transpose(1, 0, 2).reshape(128, NCH * 65)
        )
        in_maps.append({"v128": v128})
    return in_maps


def _prepare_b_in_maps(embed_table, domain_ids, p_full, msum_full):
    embed_table = np.asarray(embed_table, dtype=np.float32)
    domain_ids = np.asarray(domain_ids).astype(np.int64)
    E = np.ascontiguousarray(embed_table[domain_ids])          # [512, 2048] f32
    Ebf = E.astype(BF)
    e4 = np.ascontiguousarray(Ebf.reshape(KT, 128, S))
    ms2 = (msum_full.astype(np.float64) ** 2).astype(np.float32)
    # per-partition layout [128, 12]
    pm_pp = np.empty((128, 3 * KT), dtype=np.float32)
    pm_pp[:, 0:KT] = p_full.reshape(KT, 128).T
    pm_pp[:, KT : 2 * KT] = msum_full.reshape(KT, 128).T
    pm_pp[:, 2 * KT : 3 * KT] = ms2.reshape(KT, 128).T
    ms2_bc = np.ascontiguousarray(
        np.broadcast_to(ms2.astype(BF), (128, D))
    )
    in_maps = []
    for c in range(NCORES):
        cols = slice(c * COLS_PER_CORE, (c + 1) * COLS_PER_CORE)
        ecols = np.ascontiguousarray(
            e4[:, :, cols].transpose(1, 0, 2).reshape(128, KT * COLS_PER_CORE)
        )
        in_maps.append(
            {"pm_pp": pm_pp, "ms2_bc": ms2_bc, "ecols": ecols, "efull": e4}
        )
    return in_maps


def kernel(vulns, embed_table, domain_ids, _trace=False):
    if "nc_a" not in _CACHE:
        _CACHE["nc_a"] = build_kernel_a()
    if "nc_b" not in _CACHE:
        _CACHE["nc_b"] = build_kernel_b()

    res_a = run_bass_kernel_spmd(
        _CACHE["nc_a"], _prepare_a_in_maps(vulns),
        core_ids=list(range(NCORES)), trace=_trace,
    )
    _CACHE["res_a"] = res_a
    idx = np.arange(ROWS_PER_CORE)
    p_full = np.concatenate(
        [res_a.results[c]["out_g"][idx, idx] for c in range(NCORES)]
    )
    msum_full = np.concatenate(
        [res_a.results[c]["out_g"][64, :] for c in range(NCORES)]
    )

    res_b = run_bass_kernel_spmd(
        _CACHE["nc_b"], _prepare_b_in_maps(embed_table, domain_ids, p_full, msum_full),
        core_ids=list(range(NCORES)), trace=_trace,
    )
    _CACHE["res_b"] = res_b

    out = np.empty((S, S), dtype=np.complex64)
    for c in range(NCORES):
        r = res_b.results[c]
        sl = slice(c * COLS_PER_CORE, (c + 1) * COLS_PER_CORE)
        out[:, sl] = (
            r["out_re"].astype(np.float32).T
            + 1j * r["out_im"].astype(np.float32).T
        )
    return out


if __name__ == "__main__":
    rng = np.random.default_rng(0)
    v = rng.standard_normal((D, V), dtype=np.float32)
    et = rng.standard_normal((D, S), dtype=np.float32)
    ids = np.arange(D, dtype=np.int32)
    out = kernel(v, et, ids)
    print(out.shape, out.dtype)
